# revision 1
# baseline (speedup 1.0000x reference)
"""Trainium2 Bass kernel for an LSTM attention decoder (LAS-style).

Reference model per decoder step t (teacher forcing, L=128 steps):
    x_t   = emb[text[:, t-1]]            (t=0 -> emb[SOS])
    inp   = [x_t, ctx_{t-1}]             ctx_{-1} = values[:, 0, :]
    h1,c1 = LSTMCell1(inp, h1, c1)       H=1024
    h2,c2 = LSTMCell2(h1, h2, c2)        K=128
    energy= einsum('ntk,nk->nt', key, h2);  mask;  attn = softmax
    ctx_t = einsum('nt,ntv->nv', attn, values)
    pred_t= [h2, ctx_t] @ W_out.T + b_out

Sharding over 8 NeuronCores (SPMD — one program, per-core asymmetry only
through the input tensors):
  - LSTM1 hidden-sharded: core k owns hidden units [128k,128(k+1)) and the
    matching 512 rows of (W_ih1|W_hh1); computes its gate slice feature-major
    ([gate,batch]) for the full batch.  h1.T chunks are AllGathered per step.
  - LSTM2 replicated for the full batch (matmul cost is set by the moving
    free dim, so M=32 costs the same as M=4; no gathered-tensor slicing
    needed).
  - Attention batch-sharded via per-core key/values inputs; "my" h2 columns
    are selected with a per-core one-hot matmul.  ctx rows are AllGathered.
  - Output projection vocab-sharded and deferred: h2.T/ctx.T strips are
    banked per step; the [256, vocab/8] projection runs once at the end.
  - sigmoid(x) = 0.5*tanh(x/2)+0.5 so tanh+exp share one ACT table set.
    States are stored doubled (Hs=2h, Cs=2c) with every h-consuming weight
    pre-halved, making the sigmoid rescale free (folded into ACT scale).

Host-side numpy does memory layout only (transposes, shard slicing, dtype
casts, index shifting); all model math (matmuls, gates, softmax, embedding
gather, masking) runs on device.
"""

import numpy as np
import ml_dtypes

import concourse.bacc as bacc
import concourse.bass as bass
import concourse.mybir as mybir
import concourse.tile as tile
from concourse.bass_utils import run_bass_kernel_spmd

F32 = mybir.dt.float32
BF16 = mybir.dt.bfloat16
I32 = mybir.dt.int32
AF = mybir.ActivationFunctionType
ALU = mybir.AluOpType

NCORES = 8
N, T, L = 32, 512, 128
V, E, H, KS, VS = 8000, 256, 1024, 128, 128
NB = N // NCORES          # 4 attention rows per core
HS = H // NCORES          # 128 hidden units per core
G4 = 4 * HS               # 512 gate rows per core (i,f,g,o chunks)
KC = 11                   # K-chunks: 0,1=x(256)  2=ctx(128)  3..10=h(1024)
VS8 = V // NCORES         # 1000 vocab rows per core
VPAD = 1024               # padded vocab shard (8*128)
NEG = -1.0e9

_CACHE = {}


def _build(n_steps=L):
    nc = bacc.Bacc()

    def din(name, shape, dt=F32):
        return nc.dram_tensor(name, shape, dt, kind="ExternalInput")

    w1t = din("w1t", [128, KC * G4], BF16)        # cell1 lhsT (hh part /2)
    w2it = din("w2it", [128, 8 * G4], BF16)       # W_ih2.T/2  (moving)
    whh2t = din("whh2t", [128, G4], BF16)         # W_hh2.T/2  (moving)
    woutt = din("woutt", [128, 2 * VPAD], BF16)   # W_out.T shard (h2 part /2)
    emb_e = din("emb", [V, E], F32)               # full embedding table
    idx_e = din("idx", [N, L], I32)               # shifted token ids (SOS first)
    keyt = din("keyt", [128, NB * T], BF16)       # key[row].T  (k; n,c,tl)
    valt = din("valt", [128, NB * 4 * 128], BF16)  # values (tl; n,c,v)
    v0t = din("v0t", [128, N], BF16)              # values[:,0,:].T  ctx init
    selm = din("selm", [N, NB], F32)              # one-hot batch-row selector
    b1a = din("b1a", [128, 4], F32)               # cell1 ACT bias (pre-scaled)
    b2r = din("b2r", [1, G4], F32)                # cell2 bias row
    borr = din("borr", [1, VPAD], F32)            # b_out shard row (padded)
    lensb = din("lensb", [128, 4 * NB], I32)      # lens broadcast (c,n)
    tgrid = din("tgrid", [128, 4 * NB], I32)      # t index grid 128c+p

    out_e = nc.dram_tensor("out", [VPAD, N * n_steps], F32, kind="ExternalOutput")

    rg = [list(range(NCORES))]

    with tile.TileContext(nc) as tc:
        with (
            tc.tile_pool(name="const", bufs=1) as cst,
            tc.tile_pool(name="work", bufs=3) as wk,
            tc.tile_pool(name="state", bufs=2) as st,
            tc.tile_pool(name="psA", bufs=2, space="PSUM") as psA,
            tc.tile_pool(name="psB", bufs=2, space="PSUM") as psB,
            tc.tile_pool(name="psS", bufs=2, space="PSUM") as psS,
            tc.tile_pool(name="dram", bufs=2, space="DRAM") as dr,
        ):
            # ---------------- constants into SBUF ----------------
            c_w1t = cst.tile([128, KC * G4], BF16)
            nc.sync.dma_start(c_w1t[:], w1t[:])
            c_w1t = c_w1t.rearrange("p (k m) -> p k m", k=KC)
            c_w2it = cst.tile([128, 8 * G4], BF16)
            nc.sync.dma_start(c_w2it[:], w2it[:])
            c_w2it = c_w2it.rearrange("p (k m) -> p k m", k=8)
            c_whh2t = cst.tile([128, G4], BF16)
            nc.sync.dma_start(c_whh2t[:], whh2t[:])
            c_woutt = cst.tile([128, 2 * VPAD], BF16)
            nc.sync.dma_start(c_woutt[:], woutt[:])
            c_woutt = c_woutt.rearrange("p (k m) -> p k m", k=2)
            c_keyt = cst.tile([128, NB * T], BF16)
            nc.sync.dma_start(c_keyt[:], keyt[:])
            c_keyt = c_keyt.rearrange("p (n c t) -> p n c t", n=NB, c=4)
            c_valt = cst.tile([128, NB * 4 * 128], BF16)
            nc.sync.dma_start(c_valt[:], valt[:])
            c_valt = c_valt.rearrange("p (n c v) -> p n c v", n=NB, c=4)
            c_v0t = cst.tile([128, N], BF16)
            nc.sync.dma_start(c_v0t[:], v0t[:])
            c_sel = cst.tile([N, NB], F32)
            nc.sync.dma_start(c_sel[:], selm[:])
            c_b1a = cst.tile([128, 4], F32)
            nc.sync.dma_start(c_b1a[:], b1a[:])
            c_b2r = cst.tile([1, G4], F32)
            nc.sync.dma_start(c_b2r[:], b2r[:])
            c_borr = cst.tile([1, VPAD], F32)
            nc.sync.dma_start(c_borr[:], borr[:])

            ones_f = cst.tile([1, 512], F32)
            nc.vector.memset(ones_f[:], 1.0)
            ones_b = cst.tile([128, 1], BF16)
            nc.vector.memset(ones_b[:], 1.0)
            iop = cst.tile([128, 128], I32)
            nc.gpsimd.iota(iop[:], pattern=[[0, 128]], base=0, channel_multiplier=1)
            iof = cst.tile([128, 128], I32)
            nc.gpsimd.iota(iof[:], pattern=[[1, 128]], base=0, channel_multiplier=0)
            id_f = cst.tile([128, 128], F32)
            nc.vector.tensor_tensor(id_f[:], iop[:], iof[:], op=ALU.is_equal)
            id_b = cst.tile([128, 128], BF16)
            nc.vector.tensor_copy(id_b[:], id_f[:])

            # ---------------- mask from encoder_lens ----------------
            c_lensb = cst.tile([128, 4 * NB], I32)
            nc.sync.dma_start(c_lensb[:], lensb[:])
            c_tgrid = cst.tile([128, 4 * NB], I32)
            nc.sync.dma_start(c_tgrid[:], tgrid[:])
            m01 = cst.tile([128, 4 * NB], F32)
            nc.vector.tensor_tensor(m01[:], c_tgrid[:], c_lensb[:], op=ALU.is_ge)
            maskneg = cst.tile([128, 4 * NB], F32)
            nc.vector.tensor_scalar_mul(maskneg[:], m01[:], NEG)

            # ---------------- embedding gather + transpose ----------------
            # idx row n holds the 128 shifted tokens of batch row n; gather
            # 128 emb rows per batch row, then transpose to feature-major.
            c_idx = cst.tile([128, N], I32)
            nc.sync.dma_start(c_idx[:], idx_e.rearrange("n l -> l n"))
            embT = cst.tile([128, 2 * N * n_steps], BF16)
            embT = embT.rearrange("p (f n t) -> p f n t", f=2, n=N)
            for n in range(N):
                eg = wk.tile([128, E], F32, tag="embg")
                nc.gpsimd.indirect_dma_start(
                    out=eg[:], out_offset=None, in_=emb_e[:],
                    in_offset=bass.IndirectOffsetOnAxis(ap=c_idx[:, n:n + 1], axis=0),
                )
                for f in range(2):
                    tp = psS.tile([128, 128], F32, tag="small")
                    nc.tensor.matmul(tp[:], eg[:, 128 * f:128 * (f + 1)], id_f[:],
                                     is_transpose=True, start=True, stop=True)
                    nc.vector.tensor_copy(embT[:, f, n, 0:n_steps], tp[:, 0:n_steps])

            # ---------------- initial state ----------------
            ctxT = cst.tile([128, N], BF16)          # ctx_{t-1}.T full batch
            nc.vector.tensor_copy(ctxT[:], c_v0t[:])
            h1g0 = cst.tile([128, 8 * N], BF16)      # gathered Hs1.T zeros
            nc.vector.memset(h1g0[:], 0.0)
            h1g = h1g0.rearrange("p (c n) -> p c n", c=8)
            c1 = cst.tile([128, N], F32)             # Cs1, feature-major shard
            nc.vector.memset(c1[:], 0.0)
            c2 = cst.tile([N, 128], F32)             # Cs2, batch-major full
            nc.vector.memset(c2[:], 0.0)
            zh2 = cst.tile([128, N], BF16)           # Hs2.T for t=0
            nc.vector.memset(zh2[:], 0.0)

            s_h2t = cst.tile([128, N * n_steps], BF16)   # banked Hs2.T strips
            s_cxt = cst.tile([128, N * n_steps], BF16)   # banked ctx.T strips

            # =====================================================
            # the recurrence
            # =====================================================
            for t in range(n_steps):
                # ---- cell1: gates.T [4x128, 32], weights stationary ----
                g1 = psA.tile([128, 4 * N], F32, tag="g1")
                g1v = g1.rearrange("p (g n) -> p g n", g=4)
                korder = [0, 1] + list(range(3, 11)) + [2]   # ctx last (AG2 overlap)
                for g in range(4):
                    for j, kc in enumerate(korder):
                        if kc < 2:
                            rhs = embT[:, kc, :, t]
                        elif kc == 2:
                            rhs = ctxT[:]
                        else:
                            rhs = h1g[:, kc - 3, :]
                        nc.tensor.matmul(
                            g1v[:, g, :], c_w1t[:, kc, 128 * g:128 * (g + 1)], rhs,
                            start=(j == 0), stop=(j == len(korder) - 1))

                # ---- cell1 nonlinearity (feature-major [128, 32]) ----
                ti = wk.tile([128, N], F32, tag="t_i")
                tf = wk.tile([128, N], F32, tag="t_f")
                tg = wk.tile([128, N], F32, tag="t_g")
                to = wk.tile([128, N], F32, tag="t_o")
                nc.scalar.activation(ti[:], g1v[:, 0, :], AF.Tanh,
                                     bias=c_b1a[:, 0:1], scale=0.5)
                nc.scalar.activation(tf[:], g1v[:, 1, :], AF.Tanh,
                                     bias=c_b1a[:, 1:2], scale=0.5)
                nc.scalar.activation(tg[:], g1v[:, 2, :], AF.Tanh,
                                     bias=c_b1a[:, 2:3], scale=1.0)
                nc.scalar.activation(to[:], g1v[:, 3, :], AF.Tanh,
                                     bias=c_b1a[:, 3:4], scale=0.5)
                m1 = wk.tile([128, N], F32, tag="m1")
                nc.vector.tensor_mul(m1[:], tf[:], c1[:])       # tf*Cs
                a1 = wk.tile([128, N], F32, tag="a1")
                nc.vector.tensor_add(a1[:], c1[:], m1[:])       # (1+tf)*Cs
                m2 = wk.tile([128, N], F32, tag="m2")
                nc.vector.tensor_mul(m2[:], ti[:], tg[:])       # ti*tg
                b1t = wk.tile([128, N], F32, tag="b1t")
                nc.vector.tensor_add(b1t[:], tg[:], m2[:])      # (1+ti)*tg
                ah = wk.tile([128, N], F32, tag="ah")
                nc.vector.tensor_scalar_mul(ah[:], a1[:], 0.5)
                c1n = st.tile([128, N], F32, tag="c1")
                nc.vector.tensor_add(c1n[:], ah[:], b1t[:])     # Cs_new
                c1 = c1n
                tc1 = wk.tile([128, N], F32, tag="tc1")
                nc.scalar.activation(tc1[:], c1[:], AF.Tanh, scale=0.5)
                m3 = wk.tile([128, N], F32, tag="m3")
                nc.vector.tensor_mul(m3[:], to[:], tc1[:])
                h1l = st.tile([128, N], BF16, tag="h1l")        # Hs1.T chunk
                nc.vector.tensor_add(h1l[:], tc1[:], m3[:])

                # ---- AllGather h1 chunks ----
                agi = dr.tile([128, N], BF16, tag="agi1")
                nc.sync.dma_start(agi[:], h1l[:])
                ago = dr.tile([8 * 128, N], BF16, tag="ago1", addr_space="Shared")
                nc.gpsimd.collective_compute(
                    "AllGather", ALU.bypass, replica_groups=rg,
                    ins=[agi[:].opt()], outs=[ago[:].opt()])
                h1gt = st.tile([128, 8 * N], BF16, tag="h1g")
                h1g = h1gt.rearrange("p (c n) -> p c n", c=8)
                nc.sync.dma_start(
                    h1g, ago.rearrange("(c p) n -> p c n", p=128))

                # ---- cell2: gates [32, 512] batch-major, weights moving ----
                h2prev = zh2[:] if t == 0 else s_h2t[:, N * (t - 1):N * t]
                g2 = psB.tile([N, G4], F32, tag="g2")
                nc.tensor.matmul(g2[:], h2prev, c_whh2t[:], start=True, stop=False)
                nc.tensor.matmul(g2[:], ones_f[0:1, 0:N], c_b2r[:],
                                 start=False, stop=False)
                for kc in range(8):
                    nc.tensor.matmul(g2[:], h1g[:, kc, :], c_w2it[:, kc, :],
                                     start=False, stop=(kc == 7))

                # ---- cell2 nonlinearity (batch-major [32, 128]) ----
                t2i = wk.tile([N, 128], F32, tag="u_i")
                t2f = wk.tile([N, 128], F32, tag="u_f")
                t2g = wk.tile([N, 128], F32, tag="u_g")
                t2o = wk.tile([N, 128], F32, tag="u_o")
                nc.scalar.activation(t2i[:], g2[:, 0:128], AF.Tanh, scale=0.5)
                nc.scalar.activation(t2f[:], g2[:, 128:256], AF.Tanh, scale=0.5)
                nc.scalar.activation(t2g[:], g2[:, 256:384], AF.Tanh, scale=1.0)
                nc.scalar.activation(t2o[:], g2[:, 384:512], AF.Tanh, scale=0.5)
                n1 = wk.tile([N, 128], F32, tag="n1")
                nc.vector.tensor_mul(n1[:], t2f[:], c2[:])
                a2 = wk.tile([N, 128], F32, tag="a2")
                nc.vector.tensor_add(a2[:], c2[:], n1[:])
                n2 = wk.tile([N, 128], F32, tag="n2")
                nc.vector.tensor_mul(n2[:], t2i[:], t2g[:])
                b2t = wk.tile([N, 128], F32, tag="b2t")
                nc.vector.tensor_add(b2t[:], t2g[:], n2[:])
                a2h = wk.tile([N, 128], F32, tag="a2h")
                nc.vector.tensor_scalar_mul(a2h[:], a2[:], 0.5)
                c2n = st.tile([N, 128], F32, tag="c2")
                nc.vector.tensor_add(c2n[:], a2h[:], b2t[:])
                c2 = c2n
                tc2 = wk.tile([N, 128], F32, tag="tc2")
                nc.scalar.activation(tc2[:], c2[:], AF.Tanh, scale=0.5)
                n3 = wk.tile([N, 128], F32, tag="n3")
                nc.vector.tensor_mul(n3[:], t2o[:], tc2[:])
                h2bm = wk.tile([N, 128], F32, tag="h2bm")       # Hs2 batch-major
                nc.vector.tensor_add(h2bm[:], tc2[:], n3[:])

                # Hs2.T full batch -> strip (cell2 lhsT next step, S bank)
                h2tp = psS.tile([128, N], F32, tag="small")
                nc.tensor.matmul(h2tp[:], h2bm[:], id_f[0:N, 0:N],
                                 is_transpose=True, start=True, stop=True)
                nc.vector.tensor_copy(s_h2t[:, N * t:N * (t + 1)], h2tp[:])
                # my 4 columns of Hs2.T via one-hot selection
                h2my_ps = psS.tile([128, NB], F32, tag="small")
                nc.tensor.matmul(h2my_ps[:], h2bm[:], c_sel[:],
                                 start=True, stop=True)
                h2my = wk.tile([128, NB], BF16, tag="h2my")
                nc.vector.tensor_copy(h2my[:], h2my_ps[:])

                # ---- attention (my NB rows) ----
                en = psS.tile([128, 4 * NB], F32, tag="small")
                env = en.rearrange("p (c n) -> p c n", c=4)
                for nn in range(NB):
                    for cc in range(4):
                        nc.tensor.matmul(
                            env[:, cc, nn:nn + 1], c_keyt[:, nn, cc, :],
                            h2my[:, nn:nn + 1], start=True, stop=True)
                me = wk.tile([128, 4 * NB], F32, tag="me")
                nc.vector.tensor_add(me[:], en[:], maskneg[:])
                pe = wk.tile([128, 4 * NB], BF16, tag="pe")
                nc.scalar.activation(pe[:], me[:], AF.Exp, scale=0.5)
                pev = pe.rearrange("p (c n) -> p c n", c=4)
                sm = psS.tile([1, NB], F32, tag="small")
                for cc in range(4):
                    nc.tensor.matmul(sm[:], ones_b[:], pev[:, cc, :],
                                     start=(cc == 0), stop=(cc == 3))
                rc = wk.tile([1, NB], F32, tag="rc")
                nc.vector.reciprocal(rc[:], sm[:])
                rct = psS.tile([NB, 1], F32, tag="small")
                nc.tensor.matmul(rct[:], rc[:], ones_f[0:1, 0:1],
                                 start=True, stop=True)
                rcs = wk.tile([NB, 1], F32, tag="rcs")
                nc.vector.tensor_copy(rcs[:], rct[:])
                # ctx.T unnormalized [128(v), NB]
                cx = psS.tile([128, NB], F32, tag="small")
                for nn in range(NB):
                    for cc in range(4):
                        nc.tensor.matmul(
                            cx[:, nn:nn + 1], c_valt[:, nn, cc, :],
                            pev[:, cc, nn:nn + 1],
                            start=(cc == 0), stop=(cc == 3))
                cxs = wk.tile([128, NB], F32, tag="cxs")
                nc.vector.tensor_copy(cxs[:], cx[:])
                cxbm = psS.tile([NB, 128], F32, tag="small")
                nc.tensor.matmul(cxbm[:], cxs[:], id_f[:],
                                 is_transpose=True, start=True, stop=True)
                cxn = wk.tile([NB, 128], BF16, tag="cxn")
                nc.vector.tensor_scalar(cxn[:], cxbm[:], rcs[:, 0:1], None,
                                        op0=ALU.mult)

                # ---- AllGather ctx rows ----
                ag2i = dr.tile([NB, 128], BF16, tag="agi2")
                nc.sync.dma_start(ag2i[:], cxn[:])
                ag2o = dr.tile([N, 128], BF16, tag="ago2", addr_space="Shared")
                nc.gpsimd.collective_compute(
                    "AllGather", ALU.bypass, replica_groups=rg,
                    ins=[ag2i[:].opt()], outs=[ag2o[:].opt()])
                cxg = wk.tile([N, 128], BF16, tag="cxg")
                nc.sync.dma_start(cxg[:], ag2o[:])
                cxgt = psS.tile([128, N], BF16, tag="smallb")
                nc.tensor.matmul(cxgt[:], cxg[:], id_b[0:N, 0:N],
                                 is_transpose=True, start=True, stop=True)
                nc.vector.tensor_copy(s_cxt[:, N * t:N * (t + 1)], cxgt[:])
                ctxT = s_cxt[:, N * t:N * (t + 1)]

            # =====================================================
            # deferred output projection  pred.T [vocab shard, (t, n)]
            # =====================================================
            NT = N * n_steps
            for m in range(VPAD // 128):
                for j in range(NT // 512):
                    pp = psB.tile([128, 512], F32, tag="g2")
                    sl = slice(512 * j, 512 * (j + 1))
                    nc.tensor.matmul(pp[:], c_woutt[:, 0, 128 * m:128 * (m + 1)],
                                     s_h2t[:, sl], start=True, stop=False)
                    nc.tensor.matmul(pp[:], c_woutt[:, 1, 128 * m:128 * (m + 1)],
                                     s_cxt[:, sl], start=False, stop=False)
                    nc.tensor.matmul(pp[:], c_borr[:, 128 * m:128 * (m + 1)],
                                     ones_f[0:1, :], start=False, stop=True)
                    po = wk.tile([128, 512], F32, tag="po")
                    nc.vector.tensor_copy(po[:], pp[:])
                    nc.sync.dma_start(out_e[128 * m:128 * (m + 1), sl], po[:])

    nc.finalize()
    return nc


# --------------------------------------------------------------------------
# host-side sharding / layout prep (numpy only; no model math)
# --------------------------------------------------------------------------
def _prep(inputs, core, n_steps=L):
    key = np.asarray(inputs["key"], np.float32)
    values = np.asarray(inputs["values"], np.float32)
    lens = np.asarray(inputs["encoder_lens"]).astype(np.int32)
    text = np.asarray(inputs["text"]).astype(np.int32)
    emb = np.ascontiguousarray(np.asarray(inputs["emb"], np.float32))
    W_ih1 = np.asarray(inputs["W_ih1"], np.float32)
    W_hh1 = np.asarray(inputs["W_hh1"], np.float32)
    b1 = (np.asarray(inputs["b_ih1"], np.float32)
          + np.asarray(inputs["b_hh1"], np.float32))
    W_ih2 = np.asarray(inputs["W_ih2"], np.float32)
    W_hh2 = np.asarray(inputs["W_hh2"], np.float32)
    b2 = (np.asarray(inputs["b_ih2"], np.float32)
          + np.asarray(inputs["b_hh2"], np.float32))
    W_out = np.asarray(inputs["W_out"], np.float32)
    b_out = np.asarray(inputs["b_out"], np.float32)

    bf = ml_dtypes.bfloat16
    k = core
    rows4 = np.arange(NB * k, NB * (k + 1))
    hrows = np.concatenate([off + np.arange(128 * k, 128 * (k + 1))
                            for off in (0, H, 2 * H, 3 * H)])

    # cell1 lhsT: K order [x(256) | ctx(128) | h(1024)/2], shard rows
    W1cat = np.concatenate(
        [W_ih1[:, :E], W_ih1[:, E:], 0.5 * W_hh1], axis=1)[hrows]  # [512,1408]
    w1t = np.ascontiguousarray(
        W1cat.T.reshape(KC, 128, G4).transpose(1, 0, 2).reshape(128, KC * G4))

    w2it = np.ascontiguousarray(
        (0.5 * W_ih2.T).reshape(8, 128, G4).transpose(1, 0, 2)
        .reshape(128, 8 * G4))
    whh2t = np.ascontiguousarray(0.5 * W_hh2.T)

    WoT = W_out.T.copy()                                          # [256, 8000]
    WoT[:KS] *= 0.5
    sh = np.zeros((2 * 128, VPAD), np.float32)
    sh[:, :VS8] = WoT[:, VS8 * k:VS8 * (k + 1)]
    woutt = np.ascontiguousarray(
        sh.reshape(2, 128, VPAD).transpose(1, 0, 2).reshape(128, 2 * VPAD))
    borr = np.zeros((1, VPAD), np.float32)
    borr[0, :VS8] = b_out[VS8 * k:VS8 * (k + 1)]

    b1s = b1[hrows].reshape(4, 128) * np.array(
        [[.5], [.5], [1.], [.5]], np.float32)
    b1a = np.ascontiguousarray(b1s.T)                             # [128, 4]
    b2r = np.ascontiguousarray(b2[None, :])                       # [1, 512]

    keyt = np.ascontiguousarray(
        key[rows4].transpose(2, 0, 1).reshape(128, NB * T))
    valt = np.ascontiguousarray(
        values[rows4].reshape(NB, 4, 128, VS)
        .transpose(2, 0, 1, 3).reshape(128, NB * 4 * 128))
    v0t = np.ascontiguousarray(values[:, 0, :].T)                 # [128, 32]

    selm = np.zeros((N, NB), np.float32)
    for j in range(NB):
        selm[NB * k + j, j] = 1.0

    idx = np.empty((N, L), np.int32)
    idx[:, 0] = 1                                                 # <sos>
    idx[:, 1:] = text[:, :L - 1]
    idx = np.ascontiguousarray(idx)

    lens4 = lens[rows4]
    lensb = np.ascontiguousarray(
        np.broadcast_to(np.tile(lens4, 4)[None, :], (128, 4 * NB))).astype(np.int32)
    tgrid = np.ascontiguousarray(
        np.arange(128, dtype=np.int32)[:, None]
        + 128 * np.repeat(np.arange(4, dtype=np.int32), NB)[None, :])

    return {
        "w1t": w1t.astype(bf), "w2it": w2it.astype(bf),
        "whh2t": whh2t.astype(bf), "woutt": woutt.astype(bf),
        "emb": emb, "idx": idx,
        "keyt": keyt.astype(bf), "valt": valt.astype(bf),
        "v0t": v0t.astype(bf), "selm": selm,
        "b1a": b1a, "b2r": b2r, "borr": borr,
        "lensb": lensb, "tgrid": tgrid,
    }


def kernel(**inputs):
    n_steps = L
    if "nc" not in _CACHE:
        _CACHE["nc"] = _build(n_steps)
    nc = _CACHE["nc"]
    in_maps = [_prep(inputs, k, n_steps) for k in range(NCORES)]
    res = run_bass_kernel_spmd(nc, in_maps, core_ids=list(range(NCORES)))
    # out per core: [VPAD, (t, n)] -> its vocab shard for all (n, t)
    out = np.empty((N, n_steps, V), np.float32)
    for k in range(NCORES):
        o = res.results[k]["out"][:VS8]               # [1000, L*N]
        out[:, :, VS8 * k:VS8 * (k + 1)] = (
            o.reshape(VS8, n_steps, N).transpose(2, 1, 0))
    return out



# revision 6
# speedup vs baseline: 13.7838x; 13.7838x over previous
"""Trainium2 Bass kernel for an LSTM attention decoder (LAS-style).

Zero-collective design: fully batch-sharded over 8 cores (4 rows each),
all weights replicated. Per-core, per step t:
    gates1 = xg[t] + (W_hh1/2)@Hs1 + W_ctx@ctx        feature-major [128,(m,r)]
    Hs1,Cs1 update (doubled-state algebra, tanh-only)
    gates2 = (W_ih2/2)@Hs1 + (W_hh2/2)@Hs2 + b2       [128,(g,r)]
    Hs2,Cs2 update
    en = keyT^T@Hs2 (+mask via rank-4 matmul); pe=exp(en/2)
    ctx = (valT@pe) / sum(pe)   (transpose dance for per-row normalize)
    strips of Hs2/ctx banked; [256->vocab] projection once at the end.

Gate tile order is i,f,o,g (PyTorch i,f,g,o reordered) so one ACT covers
i,f,o (scale .5) and one covers g (scale 1). Biases b1 are folded into the
precomputed x-gates xg; b2/b_out folded via K=1 matmuls / copy-add.

Doubled states: Hs=2h, Cs=2c; weights consuming Hs are pre-halved.
sigmoid(x)=0.5 tanh(x/2)+0.5 makes all four gates tanh-based:
    Cs' = 0.5*(1+tf)*Cs + (1+ti)*tg
    Hs' = (1+to)*tanh(Cs'/2)
Attention uses Hs2 (=2*h2) with the 0.5 folded into the exp scale.
"""

import numpy as np
import ml_dtypes

import concourse.bacc as bacc
import concourse.bass as bass
import concourse.mybir as mybir
import concourse.tile as tile
from concourse.bass_utils import run_bass_kernel_spmd

F32 = mybir.dt.float32
BF16 = mybir.dt.bfloat16
I32 = mybir.dt.int32
AF = mybir.ActivationFunctionType
ALU = mybir.AluOpType

NCORES = 8
N, T, L = 32, 512, 128
V, E, H, KS, VS = 8000, 256, 1024, 128, 128
NB = N // NCORES          # 4 rows per core
NHB = H // 128            # 8 hidden blocks of cell1
NM = 4 * NHB              # 32 gate tiles (gtype-major: i,f,o,g x 8)
VT = (V + 127) // 128     # 63 vocab tiles
VPAD = VT * 128           # 8064
NEG = -1.0e9

_CACHE = {}


FP8 = mybir.dt.float8e4
H8 = False         # fp8 h-weights: no speedup on HW and 6x worse error — off


def _build(n_steps=L, h8=H8):
    nc = bacc.Bacc()

    def din(name, shape, dt=F32):
        return nc.dram_tensor(name, shape, dt, kind="ExternalInput")

    # replicated weights (same arrays for every core)
    w1ht = din("w1ht", [128, NHB * NM * 128], FP8 if h8 else BF16)
    w1ct = din("w1ct", [128, NM * 128], BF16)         # W_ctx.T tiles [m]
    w1xt = din("w1xt", [128, 2 * NM * 128], BF16)     # W_x.T tiles [kc,m]
    w2ht = din("w2ht", [128, NHB * 4 * 128], BF16)    # (W_ih2/2).T tiles [kc,g]
    w2st = din("w2st", [128, 4 * 128], BF16)          # (W_hh2/2).T tiles [g]
    b2r = din("b2r", [1, 4 * 128], F32)               # cell2 bias row
    woutt = din("woutt", [128, 2 * VT * 128], BF16)   # W_out.T tiles [kc,v]
    boutc = din("boutc", [128, VT], F32)              # b_out as per-tile cols
    b1c = din("b1c", [128, NM], F32)                  # cell1 bias cols per tile
    emb_e = din("emb", [V, E], F32)                   # full embedding table
    # per-core inputs
    idx_e = din("idx", [128, NB], I32)                # shifted token ids (t, r)
    keyt = din("keyt", [128, NB * 4 * 128], BF16)     # key.T  [k; r, cc, t]
    valt = din("valt", [128, NB * 4 * 128], BF16)     # values [t; r, cc, v]
    v0t = din("v0t", [128, NB], BF16)                 # ctx init = values[:,0,:].T
    # rank-8 decomposition of the -1e9 attention step mask (accumulated
    # straight into the energy PSUM): mask[p,(cc,r)] = mg.T @ mh
    mg = din("mg", [2 * NB, 128], BF16)
    mh = din("mh", [2 * NB, 4 * NB], BF16)

    out_e = nc.dram_tensor("out", [VPAD, NB * n_steps], F32,
                           kind="ExternalOutput")

    with tile.TileContext(nc) as tc:
        with (
            tc.tile_pool(name="const", bufs=1) as cst,
            tc.tile_pool(name="work", bufs=3) as wk,
            tc.tile_pool(name="state", bufs=2) as st,
            tc.tile_pool(name="psA", bufs=2, space="PSUM") as psA,
            tc.tile_pool(name="psU", bufs=2, space="PSUM") as psU,
            tc.tile_pool(name="psUb", bufs=2, space="PSUM") as psUb,
            tc.tile_pool(name="psP", bufs=2, space="PSUM") as psP,
        ):
            # ---------------- constants into SBUF ----------------
            c_w1ht = cst.tile([128, NHB * NM * 128], FP8 if h8 else BF16)
            nc.sync.dma_start(c_w1ht[:], w1ht[:])
            c_w1ht = c_w1ht.rearrange("p (k m c) -> p k m c", k=NHB, m=NM)
            c_w1ct = cst.tile([128, NM * 128], BF16)
            nc.sync.dma_start(c_w1ct[:], w1ct[:])
            c_w1ct = c_w1ct.rearrange("p (m c) -> p m c", m=NM)
            c_w1xt = cst.tile([128, 2 * NM * 128], BF16)
            nc.sync.dma_start(c_w1xt[:], w1xt[:])
            c_w1xt = c_w1xt.rearrange("p (k m c) -> p k m c", k=2, m=NM)
            c_w2ht = cst.tile([128, NHB * 4 * 128], BF16)
            nc.sync.dma_start(c_w2ht[:], w2ht[:])
            c_w2ht = c_w2ht.rearrange("p (k g c) -> p k g c", k=NHB, g=4)
            c_w2st = cst.tile([128, 4 * 128], BF16)
            nc.sync.dma_start(c_w2st[:], w2st[:])
            c_w2st = c_w2st.rearrange("p (g c) -> p g c", g=4)
            c_b2r = cst.tile([1, 4 * 128], F32)
            nc.sync.dma_start(c_b2r[:], b2r[:])
            c_b2r = c_b2r.rearrange("p (g c) -> p g c", g=4)
            c_woutt = cst.tile([128, 2 * VT * 128], BF16)
            nc.sync.dma_start(c_woutt[:], woutt[:])
            c_woutt = c_woutt.rearrange("p (k v c) -> p k v c", k=2, v=VT)
            c_boutc = cst.tile([128, VT], F32)
            nc.sync.dma_start(c_boutc[:], boutc[:])
            c_b1c = cst.tile([128, NM], F32)
            nc.sync.dma_start(c_b1c[:], b1c[:])
            c_keyt = cst.tile([128, NB * 4 * 128], BF16)
            nc.sync.dma_start(c_keyt[:], keyt[:])
            c_keyt = c_keyt.rearrange("p (r c t) -> p r c t", r=NB, c=4)
            c_valt = cst.tile([128, NB * 4 * 128], BF16)
            nc.sync.dma_start(c_valt[:], valt[:])
            c_valt = c_valt.rearrange("p (r c v) -> p r c v", r=NB, c=4)
            c_v0t = cst.tile([128, NB], BF16)
            nc.sync.dma_start(c_v0t[:], v0t[:])
            c_mg = cst.tile([2 * NB, 128], BF16)
            nc.sync.dma_start(c_mg[:], mg[:])
            c_mh = cst.tile([2 * NB, 4 * NB], BF16)
            nc.sync.dma_start(c_mh[:], mh[:])
            c_idx = cst.tile([128, NB], I32)
            nc.sync.dma_start(c_idx[:], idx_e[:])

            ones_f = cst.tile([1, 512], F32)
            nc.vector.memset(ones_f[:], 1.0)
            ones_b = cst.tile([128, 1], BF16)
            nc.vector.memset(ones_b[:], 1.0)
            i4 = cst.tile([NB, NB], BF16)
            nc.vector.memset(i4[:], 0.0)
            iop = cst.tile([128, 128], I32)
            nc.gpsimd.iota(iop[:], pattern=[[0, 128]], base=0,
                           channel_multiplier=1)
            iof = cst.tile([128, 128], I32)
            nc.gpsimd.iota(iof[:], pattern=[[1, 128]], base=0,
                           channel_multiplier=0)
            id_f = cst.tile([128, 128], F32)
            nc.vector.tensor_tensor(id_f[:], iop[:], iof[:], op=ALU.is_equal)
            id_b = cst.tile([128, 128], BF16)
            nc.vector.tensor_copy(id_b[:], id_f[:])
            nc.vector.tensor_copy(i4[:], id_f[0:NB, 0:NB])

            # ---------------- embedding gather + transpose ----------------
            embT_b = cst.tile([128, 2 * NB * n_steps], BF16)
            embT = embT_b.rearrange("p (f r t) -> p f r t", f=2, r=NB)
            embTf = embT_b.rearrange("p (f rt) -> p f rt", f=2)
            for r in range(NB):
                eg = wk.tile([128, E], F32, tag="embg")
                nc.gpsimd.indirect_dma_start(
                    out=eg[:], out_offset=None, in_=emb_e[:],
                    in_offset=bass.IndirectOffsetOnAxis(
                        ap=c_idx[:, r:r + 1], axis=0),
                )
                for f in range(2):
                    tp = psU.tile([128, 128], F32, tag="u")
                    nc.tensor.matmul(tp[:], eg[:, 128 * f:128 * (f + 1)],
                                     id_f[:], is_transpose=True,
                                     start=True, stop=True)
                    nc.vector.tensor_copy(embT[:, f, r, 0:n_steps],
                                          tp[:, 0:n_steps])

            # ---------------- x-gates precompute (all steps) ----------------
            # xg[p, m, r, t] = (W_x.T chunks @ embT) + b1, bf16
            xg_b = cst.tile([128, NM * NB * n_steps], BF16)
            xg = xg_b.rearrange("p (m r t) -> p m r t", m=NM, r=NB)
            xgm = xg_b.rearrange("p (m rt) -> p m rt", m=NM)
            xgt = xg_b.rearrange("p (mr t) -> p mr t", t=n_steps)
            for m in range(NM):
                pp = psP.tile([128, NB * n_steps], F32, tag="big")
                for kc in range(2):
                    nc.tensor.matmul(
                        pp[:], c_w1xt[:, kc, m, :], embTf[:, kc, :],
                        start=(kc == 0), stop=(kc == 1))
                nc.vector.tensor_scalar(
                    xgm[:, m, :], pp[:],
                    c_b1c[:, m:m + 1], None, op0=ALU.add)

            # ---------------- initial state ----------------
            ctxT = cst.tile([128, NB], BF16)      # ctx.T (real, not doubled)
            nc.vector.tensor_copy(ctxT[:], c_v0t[:])
            h1T = cst.tile([128, NHB * NB], BF16)  # Hs1.T [hb, r]
            nc.vector.memset(h1T[:], 0.0)
            h1T = h1T.rearrange("p (k r) -> p k r", k=NHB)
            c1s = cst.tile([128, NHB * NB], F32)   # Cs1 [hb, r]
            nc.vector.memset(c1s[:], 0.0)
            c2s = cst.tile([128, NB], F32)         # Cs2 [r]
            nc.vector.memset(c2s[:], 0.0)
            h2T0 = cst.tile([128, NB], BF16)       # Hs2.T zeros at t=0
            nc.vector.memset(h2T0[:], 0.0)

            s_h2t = cst.tile([128, NB * n_steps], BF16)  # Hs2.T strips [r, t]
            s_h2t = s_h2t.rearrange("p (r t) -> p r t", r=NB)
            s_cxt = cst.tile([128, NB * n_steps], BF16)  # ctx.T strips [r, t]
            s_cxt = s_cxt.rearrange("p (r t) -> p r t", r=NB)

            # =====================================================
            # the recurrence (software-pipelined: step t+1's x+h gate
            # matmuls issue during step t's cell2/attention chains; the
            # output projection is spread across steps, one 16-step
            # block behind)
            # =====================================================
            PBLK = 16
            PTPS = (VT + PBLK - 1) // PBLK     # 4 vocab tiles per step
            out_r = out_e.rearrange("v (r t) -> v r t", r=NB)

            MSPLIT = 5      # h-tiles issued before the energy matmuls

            def g1_open_a(tt, h1T_):
                g1_ = psA.tile([128, NM * NB], F32, tag="g1")
                g1v_ = g1_.rearrange("p (m r) -> p m r", m=NM)
                nc.tensor.matmul(g1_[:], id_b[:], xgt[:, :, tt],
                                 start=True, stop=False)
                for m in range(MSPLIT):
                    for kc in range(NHB):
                        nc.tensor.matmul(g1v_[:, m, :], c_w1ht[:, kc, m, :],
                                         h1T_[:, kc, :], start=False,
                                         stop=False)
                return g1_, g1v_

            MSPLIT2 = 20    # h-tiles before the B2 (post-softmax) stage

            def g1_open_b(g1v_, h1T_, m0, m1_, stop=False):
                for m in range(m0, m1_):
                    for kc in range(NHB):
                        nc.tensor.matmul(
                            g1v_[:, m, :], c_w1ht[:, kc, m, :],
                            h1T_[:, kc, :], start=False,
                            stop=(stop and m == m1_ - 1 and kc == NHB - 1))

            def g1_close(g1v_, ctxT_, stop=True):
                for m in range(NM):
                    nc.tensor.matmul(g1v_[:, m, :], c_w1ct[:, m, :], ctxT_,
                                     start=False,
                                     stop=(stop and m == NM - 1))

            def proj_tiles(vlist, t0, t1):
                w = (t1 - t0) * NB
                for v in vlist:
                    pp = psP.tile([128, 512], F32, tag="big")
                    nc.tensor.matmul(pp[:, 0:w], c_woutt[:, 0, v, :],
                                     s_h2t[:, :, t0:t1], start=True,
                                     stop=False)
                    nc.tensor.matmul(pp[:, 0:w], c_woutt[:, 1, v, :],
                                     s_cxt[:, :, t0:t1], start=False,
                                     stop=True)
                    po = wk.tile([128, 512], F32, tag="po")
                    nc.vector.tensor_scalar(po[:, 0:w], pp[:, 0:w],
                                            c_boutc[:, v:v + 1], None,
                                            op0=ALU.add)
                    nc.sync.dma_start(out_r[128 * v:128 * (v + 1), :, t0:t1],
                                      po[:, 0:w])

            nblk = n_steps // PBLK

            # prologue: gates for step 0 (h1 is zeros, ctx = values[:,0,:])
            g1, g1v = g1_open_a(0, h1T)
            g1_open_b(g1v, h1T, MSPLIT, NM)
            g1_close(g1v, ctxT[:])

            for t in range(n_steps):
                h2prev = h2T0[:] if t == 0 else s_h2t[:, :, t - 1]

                # ---- cell1 nonlinearity ----
                # cols: i=[0,32) f=[32,64) o=[64,96) g=[96,128)
                # (gates carry an extra x8 when h8; undone via ACT scale)
                gs = 0.125 if h8 else 1.0
                t1 = wk.tile([128, NM * NB], F32, tag="t1")
                nc.scalar.activation(t1[:, 0:96], g1[:, 0:96], AF.Tanh,
                                     scale=0.5 * gs)
                nc.scalar.activation(t1[:, 96:128], g1[:, 96:128], AF.Tanh,
                                     scale=1.0 * gs)
                u1 = wk.tile([128, 96], F32, tag="u1")
                nc.vector.tensor_scalar(u1[:], t1[:, 0:96], 1.0, None,
                                        op0=ALU.add)
                uf1 = wk.tile([128, 32], F32, tag="uf1")   # 0.5*(1+tf)
                nc.vector.tensor_scalar(uf1[:], t1[:, 32:64], 0.5, 0.5,
                                        op0=ALU.mult, op1=ALU.add)
                m1 = wk.tile([128, 32], F32, tag="m1")
                nc.vector.tensor_mul(m1[:], uf1[:], c1s[:])
                m2 = wk.tile([128, 32], F32, tag="m2")
                nc.vector.tensor_mul(m2[:], u1[:, 0:32], t1[:, 96:128])
                c1n = st.tile([128, NHB * NB], F32, tag="c1")
                nc.vector.tensor_add(c1n[:], m1[:], m2[:])
                c1s = c1n
                tc1 = wk.tile([128, 32], F32, tag="tc1")
                nc.scalar.activation(tc1[:], c1s[:], AF.Tanh, scale=0.5)
                h1n = st.tile([128, NHB * NB], BF16, tag="h1")
                nc.vector.tensor_mul(h1n[:], u1[:, 64:96], tc1[:])
                h1T = h1n.rearrange("p (k r) -> p k r", k=NHB)

                # ---- cell2 gates [128, (g, r)] ----
                g2u = psU.tile([128, 128], F32, tag="u")
                g2v = g2u.rearrange("p (g r) -> p g r", g=32)
                for g in range(4):
                    nc.tensor.matmul(g2v[:, g, :], c_b2r[:, g, :],
                                     ones_f[0:1, 0:NB], start=True, stop=False)
                    for kc in range(NHB):
                        nc.tensor.matmul(g2v[:, g, :], c_w2ht[:, kc, g, :],
                                         h1T[:, kc, :], start=False,
                                         stop=False)
                    nc.tensor.matmul(g2v[:, g, :], c_w2st[:, g, :], h2prev,
                                     start=False, stop=True)

                # ---- cell2 nonlinearity ----
                t2 = wk.tile([128, 4 * NB], F32, tag="t2")
                nc.scalar.activation(t2[:, 0:12], g2u[:, 0:12], AF.Tanh,
                                     scale=0.5)
                nc.scalar.activation(t2[:, 12:16], g2u[:, 12:16], AF.Tanh,
                                     scale=1.0)
                u2 = wk.tile([128, 12], F32, tag="u2")
                nc.vector.tensor_scalar(u2[:], t2[:, 0:12], 1.0, None,
                                        op0=ALU.add)
                uf2 = wk.tile([128, NB], F32, tag="uf2")   # 0.5*(1+tf)
                nc.vector.tensor_scalar(uf2[:], t2[:, 4:8], 0.5, 0.5,
                                        op0=ALU.mult, op1=ALU.add)
                n1 = wk.tile([128, NB], F32, tag="n1")
                nc.vector.tensor_mul(n1[:], uf2[:], c2s[:])
                n2 = wk.tile([128, NB], F32, tag="n2")
                nc.vector.tensor_mul(n2[:], u2[:, 0:4], t2[:, 12:16])
                c2n = st.tile([128, NB], F32, tag="c2")
                nc.vector.tensor_add(c2n[:], n1[:], n2[:])
                c2s = c2n
                tc2 = wk.tile([128, NB], F32, tag="tc2")
                nc.scalar.activation(tc2[:], c2s[:], AF.Tanh, scale=0.5)
                nc.vector.tensor_mul(s_h2t[:, :, t], u2[:, 8:12], tc2[:])
                h2T = s_h2t[:, :, t]

                # ---- open next step's gates (x + first h tiles) on PE ----
                if t + 1 < n_steps:
                    g1n, g1nv = g1_open_a(t + 1, h1T)

                # ---- attention energies (mask pre-accumulated) ----
                enu = psU.tile([128, 128], F32, tag="u")
                env = enu.rearrange("p (c r) -> p c r", c=32)
                nc.tensor.matmul(enu[:, 0:4 * NB], c_mg[:], c_mh[:],
                                 start=True, stop=False)
                for cc in range(4):
                    for r in range(NB):
                        nc.tensor.matmul(env[:, cc, r:r + 1],
                                         c_keyt[:, r, cc, :], h2T[:, r:r + 1],
                                         start=False,
                                         stop=(cc == 3 and r == NB - 1))
                pe = wk.tile([128, 4 * NB], BF16, tag="pe")
                nc.scalar.activation(pe[:], enu[:, 0:4 * NB], AF.Exp,
                                     scale=0.5)
                pev = pe.rearrange("p (c r) -> p c r", c=4)
                smu = psU.tile([128, 128], F32, tag="u")
                sm = smu[0:NB, 0:1]
                for cc in range(4):
                    nc.tensor.matmul(sm, pev[:, cc, :], ones_b[:],
                                     start=(cc == 0), stop=(cc == 3))
                rc = wk.tile([NB, 1], F32, tag="rc")
                nc.vector.reciprocal(rc[:], sm)
                cxu = psU.tile([128, 128], F32, tag="u")
                for r in range(NB):
                    for cc in range(4):
                        nc.tensor.matmul(cxu[:, r:r + 1], c_valt[:, r, cc, :],
                                         pev[:, cc, r:r + 1],
                                         start=(cc == 0), stop=(cc == 3))
                cxs = wk.tile([128, NB], BF16, tag="cxs")
                nc.vector.tensor_copy(cxs[:], cxu[:, 0:NB])
                cxbu = psUb.tile([128, 128], BF16, tag="ub")
                cxbm = cxbu[0:NB, :]
                nc.tensor.matmul(cxbm, cxs[:], id_b[:], is_transpose=True,
                                 start=True, stop=True)
                cxn = wk.tile([NB, 128], BF16, tag="cxn")
                nc.vector.tensor_scalar(cxn[:], cxbm, rc[:, 0:1], None,
                                        op0=ALU.mult)
                cxtu = psUb.tile([128, 128], BF16, tag="ub")
                cxT = cxtu[:, 0:NB]
                nc.tensor.matmul(cxT, cxn[:], i4[:], is_transpose=True,
                                 start=True, stop=True)
                nc.vector.tensor_copy(s_cxt[:, :, t], cxT)
                ctxT = s_cxt[:, :, t]

                # ---- rest of next step's h tiles ----
                # The static scheduler's cost model undercosts these 216
                # matmuls (LDWEIGHTS unmodeled), so left to itself it runs
                # them ahead of the attention chain, starving the PE tail.
                # A bypass-copy of h1 that also depends on tc2 delays their
                # readiness until the energies can issue first; the earlier-
                # priority attention PE ops then weave in.
                if t + 1 < n_steps:
                    h1b = st.tile([128, NHB * NB], BF16, tag="h1b")
                    nc.vector.tensor_scalar(h1b[:], h1n[:], tc2[:, 0:1],
                                            None, op0=ALU.bypass)
                    g1_open_b(g1nv,
                              h1b.rearrange("p (k r) -> p k r", k=NHB),
                              MSPLIT, NM)
                    g1_close(g1nv, ctxT[:])
                    g1 = g1n
                # projection slice of the previous 16-step block (covers
                # the next step's cell1 nonlinearity chain)
                blk = t // PBLK
                loc = t % PBLK
                if blk >= 1:
                    vl = range(loc * PTPS, min((loc + 1) * PTPS, VT))
                    proj_tiles(vl, (blk - 1) * PBLK, blk * PBLK)

            # =====================================================
            # projection tail: last block (+ any remainder steps)
            # =====================================================
            t0 = max(nblk - 1, 0) * PBLK
            if t0 < n_steps:
                proj_tiles(range(VT), t0, n_steps)

    nc.finalize()
    return nc


# --------------------------------------------------------------------------
# host-side sharding / layout prep (numpy only; no model math)
# --------------------------------------------------------------------------
def _prep_shared(inputs, h8=H8):
    """Replicated weight tensors (same for every core)."""
    bf = ml_dtypes.bfloat16
    f8 = ml_dtypes.float8_e4m3
    hsc = 8.0 if h8 else 1.0
    emb = np.ascontiguousarray(np.asarray(inputs["emb"], np.float32))
    W_ih1 = np.asarray(inputs["W_ih1"], np.float32)
    W_hh1 = np.asarray(inputs["W_hh1"], np.float32)
    b1 = (np.asarray(inputs["b_ih1"], np.float32)
          + np.asarray(inputs["b_hh1"], np.float32))
    W_ih2 = np.asarray(inputs["W_ih2"], np.float32)
    W_hh2 = np.asarray(inputs["W_hh2"], np.float32)
    b2 = (np.asarray(inputs["b_ih2"], np.float32)
          + np.asarray(inputs["b_hh2"], np.float32))
    W_out = np.asarray(inputs["W_out"], np.float32)
    b_out = np.asarray(inputs["b_out"], np.float32)

    # gate-tile row order: i,f,o,g (PyTorch rows i=0,f=H,g=2H,o=3H)
    goff = [0, H, 3 * H, 2 * H]
    rows = np.concatenate([off + np.arange(H) for off in goff])  # [4096]

    W1 = (hsc * W_ih1)[rows]                # [4096, 384]
    Wh1 = (hsc * 0.5 * W_hh1)[rows]         # [4096, 1024]
    b1o = (hsc * b1)[rows]                  # [4096]

    # w1ht [128, (kc 8, m 32, 128)]: tile (kc, m) = Wh1[m-rows, kc-cols].T
    # layout [p=kcol within chunk][kc][m][gate-row]
    w1ht = np.ascontiguousarray(
        Wh1.reshape(NM, 128, NHB, 128).transpose(3, 2, 0, 1)
        .reshape(128, NHB * NM * 128))

    w1ct = np.ascontiguousarray(
        W1[:, E:E + VS].reshape(NM, 128, 128).transpose(2, 0, 1)
        .reshape(128, NM * 128))
    w1xt = np.ascontiguousarray(
        W1[:, :E].reshape(NM, 128, 2, 128).transpose(3, 2, 0, 1)
        .reshape(128, 2 * NM * 128))
    b1c = np.ascontiguousarray(b1o.reshape(NM, 128).T)      # [128, NM]

    g2off = [0, KS, 3 * KS, 2 * KS]
    rows2 = np.concatenate([off + np.arange(KS) for off in g2off])  # [512]
    W2h = (0.5 * W_ih2)[rows2]              # [512, 1024]
    W2s = (0.5 * W_hh2)[rows2]              # [512, 128]
    b2o = b2[rows2]                         # [512]
    w2ht = np.ascontiguousarray(
        W2h.reshape(4, 128, NHB, 128).transpose(3, 2, 0, 1)
        .reshape(128, NHB * 4 * 128))
    w2st = np.ascontiguousarray(
        W2s.reshape(4, 128, 128).transpose(2, 0, 1).reshape(128, 4 * 128))
    b2r = np.ascontiguousarray(b2o[None, :])                # [1, 512]

    WoT = W_out.T.copy()                    # [256, 8000]
    WoT[:KS] *= 0.5                         # h2 part consumes Hs2=2*h2
    WoTp = np.zeros((256, VPAD), np.float32)
    WoTp[:, :V] = WoT
    woutt = np.ascontiguousarray(
        WoTp.reshape(2, 128, VT, 128).transpose(1, 0, 2, 3)
        .reshape(128, 2 * VT * 128))
    boutp = np.zeros((VPAD,), np.float32)
    boutp[:V] = b_out
    boutc = np.ascontiguousarray(boutp.reshape(VT, 128).T)  # [128, VT]

    return {
        "w1ht": w1ht.astype(f8 if h8 else bf),
        "w1ct": w1ct.astype(bf),
        "w1xt": w1xt.astype(bf), "w2ht": w2ht.astype(bf),
        "w2st": w2st.astype(bf), "b2r": b2r, "woutt": woutt.astype(bf),
        "boutc": boutc, "b1c": b1c, "emb": emb,
    }


def _prep_core(inputs, core):
    bf = ml_dtypes.bfloat16
    key = np.asarray(inputs["key"], np.float32)
    values = np.asarray(inputs["values"], np.float32)
    lens = np.asarray(inputs["encoder_lens"]).astype(np.int32)
    text = np.asarray(inputs["text"]).astype(np.int32)
    rows = np.arange(NB * core, NB * (core + 1))

    idx = np.empty((128, NB), np.int32)     # [t, r] shifted tokens
    idx[0, :] = 1                           # <sos>
    idx[1:, :] = text[rows, :L - 1].T
    idx = np.ascontiguousarray(idx)

    keyt = np.ascontiguousarray(
        key[rows].reshape(NB, 4, 128, KS).transpose(3, 0, 1, 2)
        .reshape(128, NB * 4 * 128))
    valt = np.ascontiguousarray(
        values[rows].reshape(NB, 4, 128, VS).transpose(2, 0, 1, 3)
        .reshape(128, NB * 4 * 128))
    v0t = np.ascontiguousarray(values[rows, 0, :].T)       # [128, NB]

    # rank-8 mask: mask[p, (cc, r)] = sum_j mg[j, p] * mh[j, (cc, r)]
    #   = -1e9 * [cc*128 + p >= len_r]
    # per row r (len = cc_l*128 + p_l):
    #   [t >= len] = [cc > cc_l] + [cc == cc_l]*[p >= p_l]
    mg = np.zeros((2 * NB, 128), np.float32)
    mh = np.zeros((2 * NB, 4, NB), np.float32)
    for j, l in enumerate(lens[rows]):
        cc_l, p_l = divmod(int(l), 128)
        mg[2 * j] = 1.0
        mg[2 * j + 1] = (np.arange(128) >= p_l)
        for cc in range(4):
            if cc > cc_l:
                mh[2 * j, cc, j] = NEG
            elif cc == cc_l:
                mh[2 * j + 1, cc, j] = NEG
    mh = mh.reshape(2 * NB, 4 * NB)

    return {
        "idx": idx, "keyt": keyt.astype(bf), "valt": valt.astype(bf),
        "v0t": v0t.astype(bf), "mg": mg.astype(bf), "mh": mh.astype(bf),
    }


def kernel(**inputs):
    n_steps = L
    if "nc" not in _CACHE:
        _CACHE["nc"] = _build(n_steps)
    nc = _CACHE["nc"]
    shared = _prep_shared(inputs)
    in_maps = [dict(shared, **_prep_core(inputs, k)) for k in range(NCORES)]
    res = run_bass_kernel_spmd(nc, in_maps, core_ids=list(range(NCORES)))
    out = np.empty((N, n_steps, V), np.float32)
    for k in range(NCORES):
        o = res.results[k]["out"][:V]                      # [8000, NB*L]
        out[NB * k:NB * (k + 1)] = (
            o.reshape(V, NB, n_steps).transpose(1, 2, 0))
    return out
